# revision 1
# baseline (speedup 1.0000x reference)
"""LSTM decoder with Luong attention on 8 Trainium2 NeuronCores.

Sharding: data-parallel over the batch dim (B=16 -> 2 per core);
LSTM/attention weights replicated on every core. The per-core program is
compiled for the Neuron devices and executed via a shard_map over the 8
cores; outputs are gathered back to full shape.
"""
import numpy as np

V, E, H = 32000, 512, 1024
B, T, S = 16, 256, 256
NCORES = 8

_COMPILED = {}


def _lstm_layer(x, W_ih, W_hh, b_ih, b_hh):
    import jax.numpy as jnp
    import jax
    from jax import lax

    b, t, _ = x.shape
    h_dim = W_hh.shape[1]
    x_proj = jnp.einsum('bti,gi->btg', x, W_ih) + b_ih + b_hh  # [B,T,4H]

    def step(carry, xt):
        h, c = carry
        gates = xt + h @ W_hh.T
        i, f, g, o = jnp.split(gates, 4, axis=-1)
        i, f, o = jax.nn.sigmoid(i), jax.nn.sigmoid(f), jax.nn.sigmoid(o)
        g = jnp.tanh(g)
        c = f * c + i * g
        h = o * jnp.tanh(c)
        return (h, c), h

    init = (jnp.zeros((b, h_dim), x.dtype), jnp.zeros((b, h_dim), x.dtype))
    (h_f, c_f), hs = lax.scan(step, init, x_proj.transpose(1, 0, 2))
    return hs.transpose(1, 0, 2), (h_f, c_f)


def _shard_body(tgt_seq, encoder_outputs, mask, embedding,
                W_ih_l0, W_hh_l0, b_ih_l0, b_hh_l0,
                W_ih_l1, W_hh_l1, b_ih_l1, b_hh_l1,
                W_attn, W_concat, b_concat):
    import jax
    import jax.numpy as jnp

    embedded = embedding[tgt_seq]                                   # [b,T,E]
    h0_seq, (h0, c0) = _lstm_layer(embedded, W_ih_l0, W_hh_l0, b_ih_l0, b_hh_l0)
    lstm_out, (h1, c1) = _lstm_layer(h0_seq, W_ih_l1, W_hh_l1, b_ih_l1, b_hh_l1)
    h_final = jnp.stack([h0, h1], axis=0)                           # [L,b,H]
    c_final = jnp.stack([c0, c1], axis=0)

    energy = jnp.einsum('bth,hk->btk', lstm_out, W_attn)
    scores = jnp.einsum('btk,bsk->bts', energy, encoder_outputs)    # [b,T,S]
    scores = jnp.where(mask[:, None, :], scores, jnp.finfo(scores.dtype).min)
    attn = jax.nn.softmax(scores, axis=-1)
    context = jnp.einsum('bts,bsh->bth', attn, encoder_outputs)

    concat = jnp.concatenate([lstm_out, context], axis=-1)
    decoder_outputs = jnp.einsum('btc,hc->bth', concat, W_concat) + b_concat
    return decoder_outputs, h_final, c_final


def _get_compiled():
    if 'fn' in _COMPILED:
        return _COMPILED['fn']
    import jax
    from jax.sharding import Mesh, PartitionSpec as P
    try:
        from jax.experimental.shard_map import shard_map
    except ImportError:
        from jax.shard_map import shard_map

    devices = jax.devices()[:NCORES]
    mesh = Mesh(np.asarray(devices), ('x',))
    batch = P('x')        # shard axis 0 (batch)
    rep = P()             # replicated

    in_specs = (batch, batch, batch,            # tgt_seq, encoder_outputs, mask
                rep,                            # embedding
                rep, rep, rep, rep,             # layer-0 weights
                rep, rep, rep, rep,             # layer-1 weights
                rep, rep, rep)                  # attention / concat weights
    out_specs = (batch, P(None, 'x'), P(None, 'x'))

    fn = jax.jit(shard_map(_shard_body, mesh=mesh,
                           in_specs=in_specs, out_specs=out_specs,
                           check_rep=False))
    _COMPILED['fn'] = fn
    return fn


def kernel(**inputs):
    import jax.numpy as jnp

    fn = _get_compiled()
    order = ["tgt_seq", "encoder_outputs", "mask", "embedding",
             "W_ih_l0", "W_hh_l0", "b_ih_l0", "b_hh_l0",
             "W_ih_l1", "W_hh_l1", "b_ih_l1", "b_hh_l1",
             "W_attn", "W_concat", "b_concat"]
    args = []
    for k in order:
        a = np.asarray(inputs[k])
        if a.dtype == np.int64:
            a = a.astype(np.int32)
        elif a.dtype == np.float64:
            a = a.astype(np.float32)
        args.append(a)
    dec, h_final, c_final = fn(*args)
    dec = np.asarray(dec)
    h_final = np.asarray(h_final)
    c_final = np.asarray(c_final)
    return dec, (h_final, c_final)
